# revision 16
# baseline (speedup 1.0000x reference)
"""Trainium2 Bass kernel: MoE gate (group-limited greedy top-k routing).

Reference computation (per token t of 16384, fp32):
    logits = x @ W.T                       # [T, 64]
    scores = softmax(logits, -1)
    group_scores = scores.reshape(T, 8, 8).max(-1)
    keep top-3 groups, mask the rest, top-6 (values+indices) of masked scores

Sharding: data-parallel over tokens. Each of the 8 cores gets a
contiguous shard of 2048 tokens and a replicated copy of W; no
collectives. Selection decisions are made on exact fp32 logits (softmax
is monotone per token), so only the output *weights* go through the
scalar-engine Exp table. The fp32 matmul chain accumulates chunks in
ascending order exactly like the reference GEMM, which keeps the logits
bit-identical to the reference and the top-k indices exact (measured
min decision margin on this data is 2.9e-7 — no reduced-precision
matmul survives that).

Streaming pipeline (per core, 4 blocks of 512 tokens):
  - x arrives host-pre-transposed as xt[b, p, j, t] = x[b*512 + t, j*128 + p];
    each block is four contiguous 1 MiB DMAs, alternating between the SP
    and ACT HWDGE rings, all issued upfront (16 MiB of x fits in SBUF)
  - W^T chunks [128h, 64e] are the stationary matmul operand (64-column
    LDWEIGHTS), x streams 512 tokens per matmul (fp32 moving-operand max)
    into a [64, 512] PSUM accumulator, chunks j=0..15 in order
  - per 128-token tile the accumulator is evacuated to SBUF and
    re-transposed — alternating between the PE (via identity) and the DVE
    (32x32 stream transpose) so neither engine serializes the tail
  - routing tail: no max-subtract for the softmax (|logit| < 4), selection
    on raw logits, top-8 staged in SBUF, one output DMA per block per
    tensor (host slices to top-6)
"""

from contextlib import ExitStack

import numpy as np

import concourse.bacc as bacc
import concourse.bass as bass
import concourse.mybir as mybir
import concourse.tile as tile
from concourse.bass_utils import run_bass_kernel_spmd
from concourse.masks import make_identity

P = 128
HIDDEN = 2048
N_EXPERTS = 64
N_GROUP = 8
EPG = N_EXPERTS // N_GROUP
TOP_K = 6
N_CORES = 8
TOKENS_TOTAL = 16384
TOKENS_PER_CORE = TOKENS_TOTAL // N_CORES
TPB = 512  # tokens per block = fp32 moving-operand max
NEG_BIG = -1.0e30

F32 = mybir.dt.float32
U32 = mybir.dt.uint32
AX = mybir.AxisListType
ALU = mybir.AluOpType
ACTF = mybir.ActivationFunctionType

PRIMER = 12


def _tail_block(nc, rt, ltp, prp, identity, accb, b, idx_stage, wts_stage):
    """Routing tail for one block's 4 tiles, emitted stage-major so each
    engine's FIFO always holds 4 independent ops (a tile-major emission
    head-of-line blocks every engine on the slowest cross-engine hop)."""
    tiles_pb = TPB // P
    lt_sb, lg, probs, den = {}, {}, {}, {}
    gsc, g8, gbias, lm, v8, we, rden = {}, {}, {}, {}, {}, {}, {}

    for g in range(tiles_pb):
        sl = slice(g * P, (g + 1) * P)
        lt_sb[g] = rt.tile([N_EXPERTS, P], F32, tag="lt_sb", name=f"lt_sb_{b}_{g}")
        if g % 2 == 0:
            nc.vector.tensor_copy(lt_sb[g][:], accb[:, sl])
        else:
            nc.scalar.copy(lt_sb[g][:], accb[:, sl])
    for g in range(tiles_pb):
        i = b * tiles_pb + g
        lg[g] = ltp.tile([P, N_EXPERTS], F32, tag="ltp", name=f"lgt_{i}")
        nc.tensor.transpose(lg[g][:], lt_sb[g][:],
                            identity[:N_EXPERTS, :N_EXPERTS])
    for g in range(tiles_pb):
        # softmax denominator over all 64 experts, from raw logits
        i = b * tiles_pb + g
        probs[g] = prp.tile([P, N_EXPERTS], F32, tag="prp", name=f"probs_{i}")
        den[g] = rt.tile([P, 1], F32, tag="den", name=f"den_{b}_{g}")
        nc.scalar.activation(probs[g][:], lg[g][:], ACTF.Exp,
                             accum_out=den[g][:])
    for g in range(tiles_pb):
        gsc[g] = rt.tile([P, N_GROUP], F32, tag="gsc", name=f"gsc_{b}_{g}")
        nc.vector.tensor_reduce(
            gsc[g][:],
            lg[g][:].rearrange("p (g e) -> p g e", g=N_GROUP),
            axis=AX.X,
            op=ALU.max,
        )
    for g in range(tiles_pb):
        g8[g] = rt.tile([P, 8], F32, tag="g8", name=f"g8_{b}_{g}")
        nc.vector.max(g8[g][:], gsc[g][:])
    for g in range(tiles_pb):
        # additive group mask: 0 for the top-3 groups, -1e30 for the rest
        gbias[g] = rt.tile([P, N_GROUP], F32, tag="gbias", name=f"gbias_{b}_{g}")
        nc.gpsimd.tensor_scalar(
            gbias[g][:],
            gsc[g][:],
            scalar1=g8[g][:, 2:3],
            scalar2=NEG_BIG,
            op0=ALU.is_lt,
            op1=ALU.mult,
        )
    for g in range(tiles_pb):
        lm[g] = rt.tile([P, N_EXPERTS], F32, tag="lm", name=f"lm_{b}_{g}")
        nc.vector.tensor_add(
            lm[g][:].rearrange("p (g e) -> p g e", g=N_GROUP),
            lg[g][:].rearrange("p (g e) -> p g e", g=N_GROUP),
            gbias[g][:].to_broadcast([P, N_GROUP, EPG]),
        )
    for g in range(tiles_pb):
        v8[g] = rt.tile([P, 8], F32, tag="v8", name=f"v8_{b}_{g}")
        nc.vector.max(v8[g][:], lm[g][:])
    for g in range(tiles_pb):
        nc.vector.max_index(idx_stage[:, g, :], v8[g][:], lm[g][:])
    for g in range(tiles_pb):
        # weights = exp(v) / sum(exp(logits)) for the 6 winners
        we[g] = rt.tile([P, 8], F32, tag="we", name=f"we_{b}_{g}")
        nc.scalar.activation(we[g][:], v8[g][:], ACTF.Exp)
    for g in range(tiles_pb):
        rden[g] = rt.tile([P, 1], F32, tag="rden", name=f"rden_{b}_{g}")
        nc.vector.reciprocal(rden[g][:], den[g][:])
    for g in range(tiles_pb):
        nc.gpsimd.tensor_scalar_mul(wts_stage[:, g, :], we[g][:], rden[g][:])


def build_moe_gate_stream2(ctx: ExitStack, tc, xt, w, idx_out, wts_out,
                           primer=PRIMER):
    """xt: [n_blocks, 128, 16, 512] f32 DRAM, xt[b,p,j,t] = x[b*512+t, j*128+p]
    idx_out/wts_out: [n_blocks, 4, 128, 8] DRAM (token b*512 + i*128 + p)."""
    nc = tc.nc
    n_blocks = xt.shape[0]
    tiles_pb = TPB // P
    n_chunks = HIDDEN // P
    JG = 4  # chunks per sub-DMA -> 1 MiB transfers
    n_sub = n_chunks // JG

    consts = ctx.enter_context(tc.tile_pool(name="consts", bufs=1))
    xpool = ctx.enter_context(tc.tile_pool(name="xin", bufs=(n_blocks - 1) * n_sub))
    xpool0 = ctx.enter_context(tc.tile_pool(name="xin0", bufs=n_chunks // 2))
    xtp = ctx.enter_context(tc.tile_pool(name="xtp", bufs=2, space="PSUM"))
    lgp = ctx.enter_context(tc.tile_pool(name="lgp", bufs=2, space="PSUM"))
    ltp = ctx.enter_context(tc.tile_pool(name="ltp", bufs=3, space="PSUM"))
    prp = ctx.enter_context(tc.tile_pool(name="prp", bufs=1, space="PSUM"))
    rt = ctx.enter_context(tc.tile_pool(name="rt", bufs=5))
    outp = ctx.enter_context(tc.tile_pool(name="outp", bufs=2))

    # w rides first on the ACT ring (it gates the stationary-operand prep —
    # behind x it would stall the first matmul by ~10us)
    w_sb = consts.tile([N_EXPERTS, HIDDEN], F32)
    nc.scalar.dma_start(w_sb[:], w)

    # all of x fits in SBUF (16 MiB): issue every sub-DMA upfront in
    # consumption order, alternating between the two HWDGE rings. Block 0
    # loads at 512 KiB granularity: the SDMA pipe takes several transfers
    # to reach line rate, and the first matmul is gated on the first piece.
    x_chunks = {}  # (b, j) -> AP of [P, TPB] for chunk j
    ring = [nc.sync, nc.scalar]
    k = 0
    for b in range(n_blocks):
        jg = 2 if b == 0 else JG
        pool = xpool0 if b == 0 else xpool
        for s in range(n_chunks // jg):
            xp = pool.tile([P, jg, TPB], F32, tag=f"xin{min(b,1)}",
                           name=f"x_{b}_{s}")
            ring[k % 2].dma_start(xp[:], xt[b, :, s * jg : (s + 1) * jg, :])
            for c in range(jg):
                x_chunks[(b, s * jg + c)] = xp[:, c, :]
            k += 1

    identity = consts.tile([P, P], F32)
    make_identity(nc, identity)

    # preload W^T: wt[p, j, e] = W[e, j*128 + p]
    wt = consts.tile([P, n_chunks, N_EXPERTS], F32)
    for j in range(n_chunks):
        pt = xtp.tile([P, P], F32, tag="xtp", name=f"wtp_{j}")
        nc.tensor.transpose(
            pt[:, :N_EXPERTS],
            w_sb[:, j * P : (j + 1) * P],
            identity[:N_EXPERTS, :N_EXPERTS],
        )
        nc.vector.tensor_copy(wt[:, j, :], pt[:, :N_EXPERTS])

    # HAM primer: dense transposes while the first x DMA streams in, so the
    # PE clock is at 2.4 GHz when the first real matmul issues.
    primer_sink = consts.tile([P, 1], F32)
    for i in range(primer):
        pp = xtp.tile([P, P], F32, tag="xtp", name=f"prime_{i}")
        nc.tensor.transpose(pp[:], identity[:], identity[:])
        if i == primer - 1:
            nc.vector.tensor_copy(primer_sink[:], pp[:, 0:1])

    for b in range(n_blocks):
        # one strictly-sequential accumulation chain (bitwise matches the
        # reference's fp32 sum order)
        acc = lgp.tile([N_EXPERTS, TPB], F32, tag="lgp", name=f"lgT_{b}")
        for j in range(n_chunks):
            nc.tensor.matmul(
                acc[:],
                wt[:, j, :],
                x_chunks.pop((b, j)),
                start=(j == 0),
                stop=(j == n_chunks - 1),
            )

        idx_stage = outp.tile([P, tiles_pb, 8], U32, tag="idxs", name=f"idxs_{b}")
        wts_stage = outp.tile([P, tiles_pb, 8], F32, tag="wtss", name=f"wtss_{b}")
        _tail_block(nc, rt, ltp, prp, identity, acc, b, idx_stage, wts_stage)

        nc.scalar.dma_start(idx_out[b].rearrange("i p k -> p i k"), idx_stage[:])
        nc.scalar.dma_start(wts_out[b].rearrange("i p k -> p i k"), wts_stage[:])


def build_nc(primer=PRIMER, num_devices=N_CORES):
    nc = bacc.Bacc(
        "TRN2",
        target_bir_lowering=False,
        debug=False,
        enable_asserts=False,
        num_devices=num_devices,
    )
    n_blocks = TOKENS_PER_CORE // TPB
    x = nc.dram_tensor(
        "x", [n_blocks, P, HIDDEN // P, TPB], F32, kind="ExternalInput"
    )
    w = nc.dram_tensor("w", [N_EXPERTS, HIDDEN], F32, kind="ExternalInput")
    idx = nc.dram_tensor(
        "idx", [n_blocks, TPB // P, P, 8], U32, kind="ExternalOutput"
    )
    wts = nc.dram_tensor(
        "wts", [n_blocks, TPB // P, P, 8], F32, kind="ExternalOutput"
    )
    with tile.TileContext(nc) as tc, ExitStack() as ctx:
        build_moe_gate_stream2(
            ctx, tc, x.ap(), w.ap(), idx.ap(), wts.ap(), primer=primer
        )
    nc.compile()
    return nc


_NC_CACHE = {}


def _get_nc():
    key = PRIMER
    if key not in _NC_CACHE:
        _NC_CACHE[key] = build_nc(primer=PRIMER)
    return _NC_CACHE[key]


def shard_stream(xs: np.ndarray) -> list[np.ndarray]:
    """Token-shard and lay out block-major:
    out[c][b, p, j, t] = xs[c*2048 + b*512 + t, j*128 + p]."""
    v = xs.reshape(N_CORES, TOKENS_PER_CORE // TPB, TPB, HIDDEN // P, P)
    v = v.transpose(0, 1, 4, 3, 2)  # [c, b, p, j, t]
    return [np.ascontiguousarray(v[c]) for c in range(N_CORES)]


def run_on_cores(xs: np.ndarray, w: np.ndarray, trace: bool = False, nc=None, **kwargs):
    """xs: [16384, 2048] f32; w: [64, 2048] f32. Returns BassKernelResults."""
    if nc is None:
        nc = _get_nc()
    shards = shard_stream(xs)
    in_maps = [{"x": shards[c], "w": w} for c in range(N_CORES)]
    return run_bass_kernel_spmd(
        nc, in_maps, core_ids=list(range(N_CORES)), trace=trace, **kwargs
    )


def kernel(x: np.ndarray, weight: np.ndarray):
    xs = np.ascontiguousarray(
        np.asarray(x, dtype=np.float32).reshape(TOKENS_TOTAL, HIDDEN)
    )
    w = np.ascontiguousarray(np.asarray(weight, dtype=np.float32))
    res = run_on_cores(xs, w)
    idx = np.concatenate(
        [r["idx"].reshape(TOKENS_PER_CORE, 8)[:, :TOP_K].astype(np.int32)
         for r in res.results],
        axis=0,
    )
    wts = np.concatenate(
        [r["wts"].reshape(TOKENS_PER_CORE, 8)[:, :TOP_K].astype(np.float32)
         for r in res.results],
        axis=0,
    )
    return idx, wts
